# revision 17
# baseline (speedup 1.0000x reference)
"""Trainium2 Bass kernel for nn_MemoryMultiAttention.

out = x + softmax((x @ Wq + bq) K^T / sqrt(D)) V   per head, tiny shared
memory bank (M=64 slots), H=4 heads of dh=16, D=64.

Strategy (v7):
  * Host folds the Q projection into the score matrix:
        scores[t, h, m] = x[t, :] @ A_h[:, m]
    with A_h = Wq_h @ K_h^T / 8 (64x64); the bias term exp(c_h[m]) is
    folded multiplicatively into the V rows (softmax is scale-invariant
    in the numerator/denominator pair), so no per-partition exp bias is
    needed and the two per-pp exps can split across engines freely.
  * Data-parallel over 8 cores; each core gets 122 chunks of 128 tokens
    (NT=15616, 8*15616=124928 >= 124800; the pad is exactly core 7's
    last chunk) = 15 full supertiles of 1024 tokens + one 256-token tail
    supertile, which also shortens the end-of-kernel drain chain.
  * The host supplies fp16 tokens (for the residual) and a bf16
    *transposed* copy laid out [128 = 2 token-halves x 64 d, cols];
    two 64-row groups of the PE run concurrently for the scores matmul.
  * Per full supertile:
      - TensorE: scoresT[hm, t] = A_pair^T @ xT   (psum [128, 2, 512] x2)
      - exp is SPLIT: ScalarE does pp0 fully + the first EB cols of each
        pp1 half; VectorE does the rest via a one-op Schraudolph exp:
        bf16_bits(exp(s)) ~= int16(round(s * 128*log2(e) + B)), emitted
        by tensor_scalar(mult, add) with an int16 output view.
      - TensorE: read_u[t, 0:64] + per-head sumexp[t, 64:68] in one
        accumulated matmul against an augmented block-diagonal V
      - DVE: reciprocal_approx_fast of sums, normalize to fp16
      - GpSimd: fp16 residual add (SBUF-only op) on the idle engine
  * Output is fp16 (rel err ~7e-4 vs the 2e-2 gate); host casts to fp32.
  * Token order inside a supertile is permuted so every DMA is >=1KB-
    contiguous per partition; the host applies the inverse permutation.
"""

import math
from contextlib import ExitStack

import ml_dtypes
import numpy as np

import concourse.bass as bass
import concourse.mybir as mybir
import concourse.tile as tile
from concourse import bacc
from concourse.bass_utils import run_bass_kernel_spmd

B, L, N, D = 16, 24, 325, 64
M, H = 64, 4
DH = D // H
TOK = B * L * N  # 124800
NCORES = 8
NCHUNK = 122  # 128-token chunks per core
NT = NCHUNK * 128  # 15616
NFULL = 15  # full supertiles of 1024 tokens
PTOK = NT - NFULL * 1024  # 256-token tail supertile (2 chunks, c=0 only)
XTCOLS = NFULL * 512 + PTOK  # 7936
TS = 1024
CH = 8  # chunks per full supertile

F32 = mybir.dt.float32
F16 = mybir.dt.float16
BF16 = mybir.dt.bfloat16
I16 = mybir.dt.int16

EB = 208  # cols of each pp1 half handled by ScalarE; rest go to VectorE
AEXP = 184.6649652  # 128 * log2(e)
BEXP = 16250.12  # 127*128 - 5.88 (centers the bf16 mantissa interp error)

# set by test.py to collect a profile
TRACE = False
LAST_RESULTS = None

_cached_nc = None


def _build_program():
    global _cached_nc
    if _cached_nc is not None:
        return _cached_nc

    nc = bacc.Bacc(
        "TRN2", target_bir_lowering=False, debug=False, num_devices=NCORES
    )
    x_in = nc.declare_dram_parameter("x", [NT, D], F16, isOutput=False)
    xt_in = nc.declare_dram_parameter("xt", [128, XTCOLS], BF16, isOutput=False)
    # all constants packed per partition: a (512B) | v (272B)
    k_in = nc.declare_dram_parameter("k", [128, 784], mybir.dt.uint8, isOutput=False)
    y_out = nc.declare_dram_parameter("y", [NT, D], F16, isOutput=True)

    with ExitStack() as ctx:
        tc = ctx.enter_context(tile.TileContext(nc))
        const_pool = ctx.enter_context(tc.tile_pool(name="const", bufs=1))
        xin_pool = ctx.enter_context(tc.tile_pool(name="xin", bufs=4))
        xt_pool = ctx.enter_context(tc.tile_pool(name="xt", bufs=4))
        exp_pool = ctx.enter_context(tc.tile_pool(name="expt", bufs=8))
        o16_pool = ctx.enter_context(tc.tile_pool(name="o16", bufs=4))
        out_pool = ctx.enter_context(tc.tile_pool(name="outp", bufs=3))
        rec_pool = ctx.enter_context(tc.tile_pool(name="recip", bufs=4))
        # psS ([128,2,512] f32) x2 and psR ([128,2,4,128] f32) are each 2 PSUM
        # banks; sharing one 4-slot pool (8 banks) lets the scheduler float
        # the spare slot to whichever side is behind
        ps_pool = ctx.enter_context(tc.tile_pool(name="ps", bufs=4, space="PSUM"))

        # constants, loaded in one DMA; engine views are bitcast slices.
        # Issued from the Scalar engine (HWDGE policy allows SP+ACT): the
        # Sync engine is still busy with NEFF preamble at this point, so
        # issuing the startup-critical transfers from ACT overlaps them.
        k_t = const_pool.tile([128, 784], mybir.dt.uint8)
        nc.scalar.dma_start(k_t[:, :], k_in[:, :])
        a_t = k_t[:, 0:512].bitcast(BF16).rearrange("p (a j) -> p a j", a=2)
        v_t = k_t[:, 512:784].bitcast(BF16).rearrange("p (a j) -> p a j", a=2)

        # dummy exp so the ACT function table loads during the DMA ramp
        # instead of serializing before the first real exp
        warm = const_pool.tile([1, 8], F32)
        nc.vector.memset(warm[:, :], 0.0)
        nc.scalar.activation(
            warm[:, :], warm[:, :], mybir.ActivationFunctionType.Exp
        )

        # software pipeline: scores/exp of supertile s are emitted before the
        # read/normalize phase of supertile s-1 so the PE starts the next
        # scores matmuls as soon as the previous exp drains.
        stage = {}  # s -> (expt pair list, x16 AP)
        outp = {}  # pair idx -> outt tile

        def read_phase(s):
            expt, x16 = stage.pop(s)
            half = s % 2

            if s == NFULL:  # 256-token tail supertile, c=0 only
                psR = ps_pool.tile([128, 2, 128], F32, tag="ps", name="psRt")
                for k in range(2):
                    for pp in range(2):
                        nc.tensor.matmul(
                            psR[:, k, 0:68],
                            expt[pp][:, 128 * k : 128 * (k + 1)],
                            v_t[:, pp, :],
                            start=(pp == 0),
                            stop=(pp == 1),
                        )
                rec = rec_pool.tile([128, 2, 4], F32, tag="rec", name="rect")
                nc.vector.reciprocal_approx_fast(
                    rec[:, :, :], psR[:, :, 64:68]
                )
                o16 = o16_pool.tile([128, 2, 4, 16], F16, tag="o16", name="o16t")
                nc.vector.tensor_mul(
                    o16[:, :, :, :],
                    psR[:, :, 0:64].rearrange("p k (h e) -> p k h e", e=16),
                    rec[:, :, :].unsqueeze(3).broadcast_to((128, 2, 4, 16)),
                )
                outt = out_pool.tile([128, 2 * D], F16, tag="outt", name="outtt")
                nc.gpsimd.tensor_add(
                    outt[:, :],
                    o16[:, :, :, :].rearrange("p k h e -> p (k h e)"),
                    x16[:, :],
                )
                nc.sync.dma_start(
                    y_out[TS * NFULL : NT, :].rearrange(
                        "(p q) d -> p (q d)", p=128
                    ),
                    outt[:, :],
                )
                return

            # read: chunk cc = 4c + k lives at psR[:, c, k, :];
            # cols 0:64 = read_u, 64:68 = per-head sumexp
            psR = ps_pool.tile([128, 2, 4, 128], F32, tag="ps", name=f"psR{s}")
            for cc in range(CH):
                c, k = cc // 4, cc % 4
                for pp in range(2):
                    nc.tensor.matmul(
                        psR[:, c, k, 0:68],
                        expt[pp][:, c, 128 * k : 128 * (k + 1)],
                        v_t[:, pp, :],
                        start=(pp == 0),
                        stop=(pp == 1),
                    )

            rec = rec_pool.tile([128, 2, 4, 4], F32, tag="rec")
            nc.vector.reciprocal_approx_fast(
                rec[:, :, :, :].rearrange("p b k h -> p (b k) h"),
                psR[:, :, :, 64:68].rearrange("p b k h -> p (b k) h"),
            )

            o16 = o16_pool.tile([128, 2, 4, 4, 16], F16, tag="o16")
            nc.vector.tensor_mul(
                o16[:, :, :, :, :],
                psR[:, :, :, 0:64].rearrange("p b k (h e) -> p b k h e", e=16),
                rec[:, :, :, :].unsqueeze(4).broadcast_to((128, 2, 4, 4, 16)),
            )

            if half == 0:
                outp[s // 2] = out_pool.tile(
                    [128, 2, CH * D], F16, tag="outt", name=f"outt{s}"
                )
            # residual add on the otherwise-idle GpSimd engine (SBUF-only op)
            nc.gpsimd.tensor_add(
                outp[s // 2][:, half],
                o16[:, :, :, :, :].rearrange("p b k h e -> p (b k h e)"),
                x16[:, :],
            )
            if s == NFULL - 1:
                # the last full supertile ships alone: half-sized final DMAs
                # issue earlier and shorten the tail
                nc.sync.dma_start(
                    y_out[TS * s : TS * (s + 1), :].rearrange(
                        "(p q) d -> p (q d)", p=128
                    ),
                    outp.pop(s // 2)[:, half],
                )
            elif half == 1:
                nc.sync.dma_start(
                    y_out[TS * (s - 1) : TS * (s + 1), :].rearrange(
                        "(u p q) d -> p u (q d)", u=2, p=128
                    ),
                    outp.pop(s // 2)[:, :, :],
                )

        x16_pair = xt_pair = None
        for s in range(NFULL + 1):
            # device token f (col of xt) = 512s + 128k + p; x/y rows are
            # host-permuted so row 1024s + 8p + 4c + k = device token f
            half = s % 2
            if s == NFULL:
                # tail supertile: 256 tokens, all in the c=0 row half
                xt15 = xt_pool.tile([128, PTOK], BF16, tag="xt", name="xt15")
                nc.sync.dma_start(xt15[:, :], xt_in[:, 512 * NFULL : XTCOLS])
                x16t = xin_pool.tile([128, 2 * D], F16, tag="x16", name="x16t")
                nc.sync.dma_start(
                    x16t[:, :],
                    x_in[TS * NFULL : NT, :].rearrange(
                        "(p q) d -> p (q d)", p=128
                    ),
                )
                expt = []
                for pp in range(2):
                    ps = ps_pool.tile(
                        [128, PTOK], F32, tag="ps", name=f"psSt_{pp}"
                    )
                    nc.tensor.matmul(
                        ps[:, :],
                        a_t[0:64, pp, :],
                        xt15[0:64, :],
                        start=True,
                        stop=True,
                    )
                    et = exp_pool.tile(
                        [128, PTOK], BF16, tag="expt", name=f"ett_{pp}"
                    )
                    expt.append(et)
                    if pp == 0:
                        nc.scalar.activation(
                            et[:, :], ps[:, :],
                            mybir.ActivationFunctionType.Exp,
                        )
                    else:
                        nc.vector.tensor_scalar(
                            et[:, :].bitcast(I16),
                            ps[:, :],
                            AEXP,
                            BEXP,
                            mybir.AluOpType.mult,
                            mybir.AluOpType.add,
                        )
                stage[s] = (expt, x16t)
                read_phase(s - 1)
                read_phase(s)
                continue

            if half == 0:
                # one DMA covers two supertiles: bigger descriptors,
                # half the sequencer issue cost; xt first (needed first)
                xt_pair = xt_pool.tile([128, 2, 512], BF16, tag="xt")
                if s == 0:
                    # split the first transfer so scores(0) starts sooner;
                    # the first slice rides the idle Scalar HWDGE engine
                    nc.scalar.dma_start(xt_pair[:, 0, 0:256], xt_in[:, 0:256])
                    nc.sync.dma_start(xt_pair[:, 0, 256:512], xt_in[:, 256:512])
                    nc.sync.dma_start(xt_pair[:, 1], xt_in[:, 512:1024])
                elif s == NFULL - 1:
                    # last full supertile has no pair partner (the tail
                    # supertile is fetched separately)
                    nc.sync.dma_start(
                        xt_pair[:, 0, :], xt_in[:, 512 * s : 512 * (s + 1)]
                    )
                else:
                    nc.sync.dma_start(
                        xt_pair[:, :, :],
                        xt_in[:, 512 * s : 512 * (s + 2)].rearrange(
                            "p (u f) -> p u f", u=2
                        ),
                    )
                x16_pair = xin_pool.tile([128, 2, CH * D], F16, tag="x16")
                if s == NFULL - 1:
                    nc.sync.dma_start(
                        x16_pair[:, 0, :],
                        x_in[TS * s : TS * (s + 1), :].rearrange(
                            "(p q) d -> p (q d)", p=128
                        ),
                    )
                else:
                    nc.sync.dma_start(
                        x16_pair[:, :, :],
                        x_in[TS * s : TS * (s + 2), :].rearrange(
                            "(u p q) d -> p u (q d)", u=2, p=128
                        ),
                    )
            x16 = x16_pair[:, half]
            xt = xt_pair[:, half]

            # scoresT: psS[pp][hm, (c, f)]
            expt = []
            psS = []
            for pp in range(2):
                ps = ps_pool.tile(
                    [128, 2, 512], F32, tag="ps", name=f"psS{s}_{pp}"
                )
                for c in range(2):
                    # s=0: split by token-col halves so the first matmuls
                    # start as soon as the first 256-col xt slice lands
                    fsp = (0, 256, 512) if s == 0 else (0, 512)
                    for fi in range(len(fsp) - 1):
                        f0, f1 = fsp[fi], fsp[fi + 1]
                        nc.tensor.matmul(
                            ps[:, c, f0:f1],
                            a_t[64 * c : 64 * (c + 1), pp, :],
                            xt[64 * c : 64 * (c + 1), f0:f1],
                            start=True,
                            stop=True,
                        )
                psS.append(ps)
                et = exp_pool.tile(
                    [128, 2, 512], BF16, tag="expt", name=f"et{s}_{pp}"
                )
                expt.append(et)
            del ps

            # exp, split across engines (no bias needed: exp(c) is in V).
            # ScalarE: all of pp0 + first EB cols of each pp1 half.
            nc.scalar.activation(
                expt[0][:, :, :],
                psS[0][:, :, :],
                mybir.ActivationFunctionType.Exp,
            )
            nc.scalar.activation(
                expt[1][:, :, 0:EB],
                psS[1][:, :, 0:EB],
                mybir.ActivationFunctionType.Exp,
            )
            # VectorE: Schraudolph bf16-exp on the remaining pp1 cols.
            nc.vector.tensor_scalar(
                expt[1][:, :, EB:512].bitcast(I16),
                psS[1][:, :, EB:512],
                AEXP,
                BEXP,
                mybir.AluOpType.mult,
                mybir.AluOpType.add,
            )
            stage[s] = (expt, x16)

            if s > 0:
                read_phase(s - 1)

    nc.compile()
    _cached_nc = nc
    return nc


def _host_constants(memory_bank, Wq, bq, Wk, bk, Wv, bv):
    mb = np.asarray(memory_bank, np.float32)
    Wq = np.asarray(Wq, np.float32)
    bq = np.asarray(bq, np.float32)
    Wk = np.asarray(Wk, np.float32)
    bk = np.asarray(bk, np.float32)
    Wv = np.asarray(Wv, np.float32)
    bv = np.asarray(bv, np.float32)

    K = mb @ Wk + bk  # [M, D]
    V = mb @ Wv + bv  # [M, D]
    scale = 1.0 / math.sqrt(D)

    # a_np[64c + d, pp, j]: A for head (2pp + j//64), slot j%64, replicated c
    a_np = np.zeros((128, 2, 128), np.float32)
    v_np = np.zeros((128, 2, 68), np.float32)
    for h in range(H):
        Kh = K[:, h * DH : (h + 1) * DH]  # [M, dh]
        Vh = V[:, h * DH : (h + 1) * DH]  # [M, dh]
        Ah = (Wq[:, h * DH : (h + 1) * DH] @ Kh.T) * scale  # [D, M]
        ch = (bq[h * DH : (h + 1) * DH] @ Kh.T) * scale  # [M]
        ech = np.exp(ch.astype(np.float64)).astype(np.float32)  # fold bias
        pp, half = h // 2, h % 2
        for c in range(2):
            a_np[64 * c : 64 * (c + 1), pp, 64 * half : 64 * (half + 1)] = Ah
        q0 = 64 * half
        v_np[q0 : q0 + 64, pp, h * DH : (h + 1) * DH] = Vh * ech[:, None]
        v_np[q0 : q0 + 64, pp, 64 + h] = ech

    return (
        a_np.astype(ml_dtypes.bfloat16),
        v_np.astype(ml_dtypes.bfloat16),
    )


def kernel(x, memory_bank, Wq, bq, Wk, bk, Wv, bv):
    global LAST_RESULTS
    a_np, v_np = _host_constants(memory_bank, Wq, bq, Wk, bk, Wv, bv)

    x_np = np.asarray(x, np.float32).reshape(TOK, D)
    x_pad = np.zeros((NCORES * NT, D), np.float32)
    x_pad[:TOK] = x_np
    ch = x_pad.reshape(NCORES, NCHUNK, 128, D)  # [n, ci, p, d]
    chf = ch[:, : NFULL * 8].reshape(NCORES, NFULL, 2, 4, 128, D)  # [n,s,c,k,p,d]
    cht = ch[:, NFULL * 8 :]  # [n, 2, 128, d] (k, p, d)

    # device-permuted fp16 tokens: full row 1024s + 8p + 4c + k,
    # tail row 15360 + 2p + k
    x_perm = np.empty((NCORES, NT, D), np.float16)
    x_perm[:, : NFULL * 1024] = (
        chf.transpose(0, 1, 4, 2, 3, 5).reshape(NCORES, NFULL * 1024, D)
    )
    x_perm[:, NFULL * 1024 :] = cht.transpose(0, 2, 1, 3).reshape(
        NCORES, PTOK, D
    )
    # transposed bf16 tokens: xt[n, 64c + d, 512s + 128k + p] (full),
    # tail cols 7680 + 128k + p in the c=0 row half
    xt16 = np.zeros((NCORES, 128, XTCOLS), ml_dtypes.bfloat16)
    xt16[:, :, : NFULL * 512] = (
        chf.astype(ml_dtypes.bfloat16)
        .transpose(0, 2, 5, 1, 3, 4)
        .reshape(NCORES, 128, NFULL * 512)
    )
    xt16[:, 0:64, NFULL * 512 :] = (
        cht.astype(ml_dtypes.bfloat16)
        .transpose(0, 3, 1, 2)
        .reshape(NCORES, 64, PTOK)
    )

    k_np = np.concatenate(
        [
            a_np.reshape(128, 256).view(np.uint8),
            v_np.reshape(128, 136).view(np.uint8),
        ],
        axis=1,
    )
    in_maps = [
        {"x": np.ascontiguousarray(x_perm[n]), "xt": np.ascontiguousarray(xt16[n]), "k": k_np}
        for n in range(NCORES)
    ]

    nc = _build_program()
    res = run_bass_kernel_spmd(nc, in_maps, list(range(NCORES)), trace=TRACE)
    LAST_RESULTS = res

    y = np.stack([res.results[n]["y"] for n in range(NCORES)], axis=0)
    y = y.astype(np.float32)
    # invert the permutation back to chunk order
    yc = np.empty((NCORES, NCHUNK, 128, D), np.float32)
    yc[:, : NFULL * 8] = (
        y[:, : NFULL * 1024]
        .reshape(NCORES, NFULL, 128, 2, 4, D)
        .transpose(0, 1, 3, 4, 2, 5)
        .reshape(NCORES, NFULL * 8, 128, D)
    )
    yc[:, NFULL * 8 :] = (
        y[:, NFULL * 1024 :]
        .reshape(NCORES, 128, 2, D)
        .transpose(0, 2, 1, 3)
    )
    return yc.reshape(NCORES * NT, D)[:TOK].reshape(B, L, N, D)


# revision 22
# speedup vs baseline: 1.0008x; 1.0008x over previous
"""Trainium2 Bass kernel for nn_MemoryMultiAttention.

out = x + softmax((x @ Wq + bq) K^T / sqrt(D)) V   per head, tiny shared
memory bank (M=64 slots), H=4 heads of dh=16, D=64.

Strategy (v7):
  * Host folds the Q projection into the score matrix:
        scores[t, h, m] = x[t, :] @ A_h[:, m]
    with A_h = Wq_h @ K_h^T / 8 (64x64); the bias term exp(c_h[m]) is
    folded multiplicatively into the V rows (softmax is scale-invariant
    in the numerator/denominator pair), so no per-partition exp bias is
    needed and the two per-pp exps can split across engines freely.
  * Data-parallel over 8 cores; each core gets 122 chunks of 128 tokens
    (NT=15616, 8*15616=124928 >= 124800; the pad is exactly core 7's
    last chunk) = 15 full supertiles of 1024 tokens + one 256-token tail
    supertile, which also shortens the end-of-kernel drain chain.
  * The host supplies fp16 tokens (for the residual) and a bf16
    *transposed* copy laid out [128 = 2 token-halves x 64 d, cols];
    two 64-row groups of the PE run concurrently for the scores matmul.
  * Per full supertile:
      - TensorE: scoresT[hm, t] = A_pair^T @ xT   (psum [128, 2, 512] x2)
      - exp is SPLIT: ScalarE does pp0 fully + the first EB cols of each
        pp1 half; VectorE does the rest via a one-op Schraudolph exp:
        bf16_bits(exp(s)) ~= int16(round(s * 128*log2(e) + B)), emitted
        by tensor_scalar(mult, add) with an int16 output view.
      - TensorE: read_u[t, 0:64] + per-head sumexp[t, 64:68] in one
        accumulated matmul against an augmented block-diagonal V
      - DVE: reciprocal_approx_fast of sums, normalize to fp16
      - GpSimd: fp16 residual add (SBUF-only op) on the idle engine
  * Output is fp16 (rel err ~7e-4 vs the 2e-2 gate); host casts to fp32.
  * Token order inside a supertile is permuted so every DMA is >=1KB-
    contiguous per partition; the host applies the inverse permutation.
"""

import math
from contextlib import ExitStack

import ml_dtypes
import numpy as np

import concourse.bass as bass
import concourse.mybir as mybir
import concourse.tile as tile
from concourse import bacc
from concourse.bass_utils import run_bass_kernel_spmd

B, L, N, D = 16, 24, 325, 64
M, H = 64, 4
DH = D // H
TOK = B * L * N  # 124800
NCORES = 8
NCHUNK = 122  # 128-token chunks per core
NT = NCHUNK * 128  # 15616
NFULL = 15  # full supertiles of 1024 tokens
PTOK = NT - NFULL * 1024  # 256-token tail supertile (2 chunks, c=0 only)
XTCOLS = NFULL * 512 + PTOK  # 7936
TS = 1024
CH = 8  # chunks per full supertile

F32 = mybir.dt.float32
F16 = mybir.dt.float16
BF16 = mybir.dt.bfloat16
I16 = mybir.dt.int16

EB = 176  # cols of each pp1 half handled by ScalarE; rest go to VectorE
AEXP = 184.6649652  # 128 * log2(e)
BEXP = 16250.12  # 127*128 - 5.88 (centers the bf16 mantissa interp error)

# set by test.py to collect a profile
TRACE = False
LAST_RESULTS = None

_cached_nc = None


def _build_program():
    global _cached_nc
    if _cached_nc is not None:
        return _cached_nc

    nc = bacc.Bacc(
        "TRN2", target_bir_lowering=False, debug=False, num_devices=NCORES
    )
    x_in = nc.declare_dram_parameter("x", [NT, D], F16, isOutput=False)
    xt_in = nc.declare_dram_parameter("xt", [128, XTCOLS], BF16, isOutput=False)
    # all constants packed per partition: a (512B) | v (272B)
    k_in = nc.declare_dram_parameter("k", [128, 784], mybir.dt.uint8, isOutput=False)
    y_out = nc.declare_dram_parameter("y", [NT, D], F16, isOutput=True)

    with ExitStack() as ctx:
        tc = ctx.enter_context(tile.TileContext(nc))
        const_pool = ctx.enter_context(tc.tile_pool(name="const", bufs=1))
        xin_pool = ctx.enter_context(tc.tile_pool(name="xin", bufs=4))
        xt_pool = ctx.enter_context(tc.tile_pool(name="xt", bufs=4))
        exp_pool = ctx.enter_context(tc.tile_pool(name="expt", bufs=6))
        o16_pool = ctx.enter_context(tc.tile_pool(name="o16", bufs=3))
        out_pool = ctx.enter_context(tc.tile_pool(name="outp", bufs=3))
        rec_pool = ctx.enter_context(tc.tile_pool(name="recip", bufs=3))
        # psS ([128,2,512] f32) x2 and psR ([128,2,4,128] f32) are each 2 PSUM
        # banks; sharing one 4-slot pool (8 banks) lets the scheduler float
        # the spare slot to whichever side is behind
        ps_pool = ctx.enter_context(tc.tile_pool(name="ps", bufs=4, space="PSUM"))

        # constants, loaded in one DMA; engine views are bitcast slices.
        # Issued from the Scalar engine (HWDGE policy allows SP+ACT): the
        # Sync engine is still busy with NEFF preamble at this point, so
        # issuing the startup-critical transfers from ACT overlaps them.
        k_t = const_pool.tile([128, 784], mybir.dt.uint8)
        nc.scalar.dma_start(k_t[:, :], k_in[:, :])
        a_t = k_t[:, 0:512].bitcast(BF16).rearrange("p (a j) -> p a j", a=2)
        v_t = k_t[:, 512:784].bitcast(BF16).rearrange("p (a j) -> p a j", a=2)

        # dummy exp so the ACT function table loads during the DMA ramp
        # instead of serializing before the first real exp
        warm = const_pool.tile([1, 8], F32)
        nc.vector.memset(warm[:, :], 0.0)
        nc.scalar.activation(
            warm[:, :], warm[:, :], mybir.ActivationFunctionType.Exp
        )

        # software pipeline: scores/exp of supertile s are emitted before the
        # read/normalize phase of supertile s-1 so the PE starts the next
        # scores matmuls as soon as the previous exp drains.
        stage = {}  # s -> (expt pair list, x16 AP)
        outp = {}  # pair idx -> outt tile

        def read_phase(s):
            expt, x16 = stage.pop(s)
            half = s % 2

            if s == NFULL:  # 256-token tail supertile, c=0 only
                psR = ps_pool.tile([128, 2, 128], F32, tag="ps", name="psRt")
                for k in range(2):
                    for pp in range(2):
                        nc.tensor.matmul(
                            psR[:, k, 0:68],
                            expt[pp][:, 128 * k : 128 * (k + 1)],
                            v_t[:, pp, :],
                            start=(pp == 0),
                            stop=(pp == 1),
                        )
                rec = rec_pool.tile([128, 2, 4], F32, tag="rec", name="rect")
                nc.vector.reciprocal_approx_fast(
                    rec[:, :, :], psR[:, :, 64:68]
                )
                o16 = o16_pool.tile([128, 2, 4, 16], F16, tag="o16", name="o16t")
                nc.vector.tensor_mul(
                    o16[:, :, :, :],
                    psR[:, :, 0:64].rearrange("p k (h e) -> p k h e", e=16),
                    rec[:, :, :].unsqueeze(3).broadcast_to((128, 2, 4, 16)),
                )
                outt = out_pool.tile([128, 2 * D], F16, tag="outt", name="outtt")
                nc.gpsimd.tensor_add(
                    outt[:, :],
                    o16[:, :, :, :].rearrange("p k h e -> p (k h e)"),
                    x16[:, :],
                )
                nc.sync.dma_start(
                    y_out[TS * NFULL : NT, :].rearrange(
                        "(p q) d -> p (q d)", p=128
                    ),
                    outt[:, :],
                )
                return

            # read: chunk cc = 4c + k lives at psR[:, c, k, :];
            # cols 0:64 = read_u, 64:68 = per-head sumexp
            psR = ps_pool.tile([128, 2, 4, 128], F32, tag="ps", name=f"psR{s}")
            for cc in range(CH):
                c, k = cc // 4, cc % 4
                for pp in range(2):
                    nc.tensor.matmul(
                        psR[:, c, k, 0:68],
                        expt[pp][:, c, 128 * k : 128 * (k + 1)],
                        v_t[:, pp, :],
                        start=(pp == 0),
                        stop=(pp == 1),
                    )

            rec = rec_pool.tile([128, 2, 4, 4], F32, tag="rec")
            nc.vector.reciprocal_approx_fast(
                rec[:, :, :, :].rearrange("p b k h -> p (b k) h"),
                psR[:, :, :, 64:68].rearrange("p b k h -> p (b k) h"),
            )

            o16 = o16_pool.tile([128, 2, 4, 4, 16], F16, tag="o16")
            nc.vector.tensor_mul(
                o16[:, :, :, :, :],
                psR[:, :, :, 0:64].rearrange("p b k (h e) -> p b k h e", e=16),
                rec[:, :, :, :].unsqueeze(4).broadcast_to((128, 2, 4, 4, 16)),
            )

            if half == 0:
                outp[s // 2] = out_pool.tile(
                    [128, 2, CH * D], F16, tag="outt", name=f"outt{s}"
                )
            # residual add on the otherwise-idle GpSimd engine (SBUF-only op)
            nc.gpsimd.tensor_add(
                outp[s // 2][:, half],
                o16[:, :, :, :, :].rearrange("p b k h e -> p (b k h e)"),
                x16[:, :],
            )
            if s == NFULL - 1:
                # the last full supertile ships alone: half-sized final DMAs
                # issue earlier and shorten the tail
                nc.sync.dma_start(
                    y_out[TS * s : TS * (s + 1), :].rearrange(
                        "(p q) d -> p (q d)", p=128
                    ),
                    outp.pop(s // 2)[:, half],
                )
            elif half == 1:
                nc.sync.dma_start(
                    y_out[TS * (s - 1) : TS * (s + 1), :].rearrange(
                        "(u p q) d -> p u (q d)", u=2, p=128
                    ),
                    outp.pop(s // 2)[:, :, :],
                )

        x16_pair = xt_pair = None
        xt_pf = {}  # pair idx -> prefetched xt tile
        for s in range(NFULL + 1):
            # device token f (col of xt) = 512s + 128k + p; x/y rows are
            # host-permuted so row 1024s + 8p + 4c + k = device token f
            half = s % 2
            if s == NFULL:
                # tail supertile: 256 tokens, all in the c=0 row half
                xt15 = xt_pool.tile([128, PTOK], BF16, tag="xt", name="xt15")
                nc.sync.dma_start(xt15[:, :], xt_in[:, 512 * NFULL : XTCOLS])
                x16t = xin_pool.tile([128, 2 * D], F16, tag="x16", name="x16t")
                nc.sync.dma_start(
                    x16t[:, :],
                    x_in[TS * NFULL : NT, :].rearrange(
                        "(p q) d -> p (q d)", p=128
                    ),
                )
                expt = []
                for pp in range(2):
                    ps = ps_pool.tile(
                        [128, PTOK], F32, tag="ps", name=f"psSt_{pp}"
                    )
                    nc.tensor.matmul(
                        ps[:, :],
                        a_t[0:64, pp, :],
                        xt15[0:64, :],
                        start=True,
                        stop=True,
                    )
                    et = exp_pool.tile(
                        [128, PTOK], BF16, tag="expt", name=f"ett_{pp}"
                    )
                    expt.append(et)
                    if pp == 0:
                        nc.scalar.activation(
                            et[:, :], ps[:, :],
                            mybir.ActivationFunctionType.Exp,
                        )
                    else:
                        nc.vector.tensor_scalar(
                            et[:, :].bitcast(I16),
                            ps[:, :],
                            AEXP,
                            BEXP,
                            mybir.AluOpType.mult,
                            mybir.AluOpType.add,
                        )
                stage[s] = (expt, x16t)
                read_phase(s - 1)
                read_phase(s)
                continue

            if half == 0:
                # one DMA covers two supertiles: bigger descriptors,
                # half the sequencer issue cost; xt first (needed first)
                prefetched = s // 2 in xt_pf
                if prefetched:
                    xt_pair = xt_pf.pop(s // 2)
                else:
                    xt_pair = xt_pool.tile([128, 2, 512], BF16, tag="xt")
                if prefetched:
                    pass
                elif s == 0:
                    # split the first transfer so scores(0) starts sooner;
                    # the first slice rides the idle Scalar HWDGE engine
                    nc.scalar.dma_start(xt_pair[:, 0, 0:256], xt_in[:, 0:256])
                    nc.sync.dma_start(xt_pair[:, 0, 256:512], xt_in[:, 256:512])
                    nc.sync.dma_start(xt_pair[:, 1], xt_in[:, 512:1024])
                    # the Scalar engine idles from here until its first exp:
                    # prefetch the next two xt pairs on it so early supertiles
                    # never stall on input DMA behind the Sync issue queue
                    for p in (1, 2):
                        pf = xt_pool.tile(
                            [128, 2, 512], BF16, tag="xt", name=f"xtpf{p}"
                        )
                        nc.scalar.dma_start(
                            pf[:, :, :],
                            xt_in[:, 1024 * p : 1024 * (p + 1)].rearrange(
                                "p (u f) -> p u f", u=2
                            ),
                        )
                        xt_pf[p] = pf
                elif s == NFULL - 1:
                    # last full supertile has no pair partner (the tail
                    # supertile is fetched separately)
                    nc.sync.dma_start(
                        xt_pair[:, 0, :], xt_in[:, 512 * s : 512 * (s + 1)]
                    )
                else:
                    nc.sync.dma_start(
                        xt_pair[:, :, :],
                        xt_in[:, 512 * s : 512 * (s + 2)].rearrange(
                            "p (u f) -> p u f", u=2
                        ),
                    )
                x16_pair = xin_pool.tile([128, 2, CH * D], F16, tag="x16")
                if s == NFULL - 1:
                    nc.sync.dma_start(
                        x16_pair[:, 0, :],
                        x_in[TS * s : TS * (s + 1), :].rearrange(
                            "(p q) d -> p (q d)", p=128
                        ),
                    )
                else:
                    nc.sync.dma_start(
                        x16_pair[:, :, :],
                        x_in[TS * s : TS * (s + 2), :].rearrange(
                            "(u p q) d -> p u (q d)", u=2, p=128
                        ),
                    )
            x16 = x16_pair[:, half]
            xt = xt_pair[:, half]

            # scoresT: psS[pp][hm, (c, f)]
            expt = []
            psS = []
            for pp in range(2):
                ps = ps_pool.tile(
                    [128, 2, 512], F32, tag="ps", name=f"psS{s}_{pp}"
                )
                for c in range(2):
                    # s=0: split by token-col halves so the first matmuls
                    # start as soon as the first 256-col xt slice lands
                    fsp = (0, 256, 512) if s == 0 else (0, 512)
                    for fi in range(len(fsp) - 1):
                        f0, f1 = fsp[fi], fsp[fi + 1]
                        nc.tensor.matmul(
                            ps[:, c, f0:f1],
                            a_t[64 * c : 64 * (c + 1), pp, :],
                            xt[64 * c : 64 * (c + 1), f0:f1],
                            start=True,
                            stop=True,
                        )
                psS.append(ps)
                et = exp_pool.tile(
                    [128, 2, 512], BF16, tag="expt", name=f"et{s}_{pp}"
                )
                expt.append(et)
            del ps

            # exp, split across engines (no bias needed: exp(c) is in V).
            # ScalarE: all of pp0 + first EB cols of each pp1 half.
            nc.scalar.activation(
                expt[0][:, :, :],
                psS[0][:, :, :],
                mybir.ActivationFunctionType.Exp,
            )
            nc.scalar.activation(
                expt[1][:, :, 0:EB],
                psS[1][:, :, 0:EB],
                mybir.ActivationFunctionType.Exp,
            )
            # VectorE: Schraudolph bf16-exp on the remaining pp1 cols.
            nc.vector.tensor_scalar(
                expt[1][:, :, EB:512].bitcast(I16),
                psS[1][:, :, EB:512],
                AEXP,
                BEXP,
                mybir.AluOpType.mult,
                mybir.AluOpType.add,
            )
            stage[s] = (expt, x16)

            if s > 0:
                read_phase(s - 1)

    nc.compile()
    _cached_nc = nc
    return nc


def _host_constants(memory_bank, Wq, bq, Wk, bk, Wv, bv):
    mb = np.asarray(memory_bank, np.float32)
    Wq = np.asarray(Wq, np.float32)
    bq = np.asarray(bq, np.float32)
    Wk = np.asarray(Wk, np.float32)
    bk = np.asarray(bk, np.float32)
    Wv = np.asarray(Wv, np.float32)
    bv = np.asarray(bv, np.float32)

    K = mb @ Wk + bk  # [M, D]
    V = mb @ Wv + bv  # [M, D]
    scale = 1.0 / math.sqrt(D)

    # a_np[64c + d, pp, j]: A for head (2pp + j//64), slot j%64, replicated c
    a_np = np.zeros((128, 2, 128), np.float32)
    v_np = np.zeros((128, 2, 68), np.float32)
    for h in range(H):
        Kh = K[:, h * DH : (h + 1) * DH]  # [M, dh]
        Vh = V[:, h * DH : (h + 1) * DH]  # [M, dh]
        Ah = (Wq[:, h * DH : (h + 1) * DH] @ Kh.T) * scale  # [D, M]
        ch = (bq[h * DH : (h + 1) * DH] @ Kh.T) * scale  # [M]
        ech = np.exp(ch.astype(np.float64)).astype(np.float32)  # fold bias
        pp, half = h // 2, h % 2
        for c in range(2):
            a_np[64 * c : 64 * (c + 1), pp, 64 * half : 64 * (half + 1)] = Ah
        q0 = 64 * half
        v_np[q0 : q0 + 64, pp, h * DH : (h + 1) * DH] = Vh * ech[:, None]
        v_np[q0 : q0 + 64, pp, 64 + h] = ech

    return (
        a_np.astype(ml_dtypes.bfloat16),
        v_np.astype(ml_dtypes.bfloat16),
    )


def kernel(x, memory_bank, Wq, bq, Wk, bk, Wv, bv):
    global LAST_RESULTS
    a_np, v_np = _host_constants(memory_bank, Wq, bq, Wk, bk, Wv, bv)

    x_np = np.asarray(x, np.float32).reshape(TOK, D)
    x_pad = np.zeros((NCORES * NT, D), np.float32)
    x_pad[:TOK] = x_np
    ch = x_pad.reshape(NCORES, NCHUNK, 128, D)  # [n, ci, p, d]
    chf = ch[:, : NFULL * 8].reshape(NCORES, NFULL, 2, 4, 128, D)  # [n,s,c,k,p,d]
    cht = ch[:, NFULL * 8 :]  # [n, 2, 128, d] (k, p, d)

    # device-permuted fp16 tokens: full row 1024s + 8p + 4c + k,
    # tail row 15360 + 2p + k
    x_perm = np.empty((NCORES, NT, D), np.float16)
    x_perm[:, : NFULL * 1024] = (
        chf.transpose(0, 1, 4, 2, 3, 5).reshape(NCORES, NFULL * 1024, D)
    )
    x_perm[:, NFULL * 1024 :] = cht.transpose(0, 2, 1, 3).reshape(
        NCORES, PTOK, D
    )
    # transposed bf16 tokens: xt[n, 64c + d, 512s + 128k + p] (full),
    # tail cols 7680 + 128k + p in the c=0 row half
    xt16 = np.zeros((NCORES, 128, XTCOLS), ml_dtypes.bfloat16)
    xt16[:, :, : NFULL * 512] = (
        chf.astype(ml_dtypes.bfloat16)
        .transpose(0, 2, 5, 1, 3, 4)
        .reshape(NCORES, 128, NFULL * 512)
    )
    xt16[:, 0:64, NFULL * 512 :] = (
        cht.astype(ml_dtypes.bfloat16)
        .transpose(0, 3, 1, 2)
        .reshape(NCORES, 64, PTOK)
    )

    k_np = np.concatenate(
        [
            a_np.reshape(128, 256).view(np.uint8),
            v_np.reshape(128, 136).view(np.uint8),
        ],
        axis=1,
    )
    in_maps = [
        {"x": np.ascontiguousarray(x_perm[n]), "xt": np.ascontiguousarray(xt16[n]), "k": k_np}
        for n in range(NCORES)
    ]

    nc = _build_program()
    res = run_bass_kernel_spmd(nc, in_maps, list(range(NCORES)), trace=TRACE)
    LAST_RESULTS = res

    y = np.stack([res.results[n]["y"] for n in range(NCORES)], axis=0)
    y = y.astype(np.float32)
    # invert the permutation back to chunk order
    yc = np.empty((NCORES, NCHUNK, 128, D), np.float32)
    yc[:, : NFULL * 8] = (
        y[:, : NFULL * 1024]
        .reshape(NCORES, NFULL, 128, 2, 4, D)
        .transpose(0, 1, 3, 4, 2, 5)
        .reshape(NCORES, NFULL * 8, 128, D)
    )
    yc[:, NFULL * 8 :] = (
        y[:, NFULL * 1024 :]
        .reshape(NCORES, 128, 2, D)
        .transpose(0, 2, 1, 3)
    )
    return yc.reshape(NCORES * NT, D)[:TOK].reshape(B, L, N, D)
